# revision 42
# baseline (speedup 1.0000x reference)
"""CapsuleConv2d (3-iteration dynamic routing) Bass kernel for 8 TRN2 cores.

Strategy (data-parallel over batch, 2 images per core):
  - priors[l, ij, o, u, f] computed by PE per 128-location tile in fp32
    (one merged-row matmul per kernel tap: strided stationary
    [32, 2 rows, 64] = 128 locs).  s0 = 0.25*sum P accumulated by PE in
    the same pass.
  - routing in "natural" layout (locations on partitions).  The big
    elementwise multiplies run on DVE in fp16 which engages the 2x_1p
    perf mode (2 elem/cycle/lane; requires every operand 2-byte with a
    packed last dim).  Segmented reductions are binary trees of
    tensor_tensor adds (also 2x in fp16) instead of tensor_reduce (which
    has no perf mode).  Softmax normalizers use the 1-cyc/elem
    reciprocal_approx_fast custom DVE op.
  - the iteration-0 H-chain (H = P*v0, tree-reduce to logits b1) runs on
    the Pool engine in fp32: removes its fp16 rounding (the dominant
    error term) AND its DVE load.  The kernel is software-pipelined:
    produce(k+1) = priors + squash0 + Pool chain is emitted before
    consume(k) = DVE routing, so Pool(k+1) overlaps DVE(k).
  - exp/ln/square and PSUM->SBUF copies on ACT; PE transposes the final
    [128 locs, 32 ch] result for channel-major DMA out.
"""
import numpy as np

import concourse.bass as bass
import concourse.bacc as bacc
import concourse.tile as tile
import concourse.mybir as mybir
import concourse.bass_utils as bass_utils

# All ACT functions we use (Exp, Ln, Square, Copy, ...) live together in the
# "natural_log_exp_and_others" table set, but bacc's table-load pass picks a
# per-function set greedily (Ln -> natural_log, Exp -> exp_and_others),
# thrashing ~2.7us table loads between them.  Restrict Exp/Ln to the combined
# set so a single load covers the whole kernel.
_orig_get_tables = bacc.get_activation_tables
_AFT = mybir.ActivationFunctionType


def _patched_get_tables(arch):
    tables = dict(_orig_get_tables(arch))
    for name, funcs in tables.items():
        if name != "natural_log_exp_and_others":
            tables[name] = funcs - {_AFT.Exp, _AFT.Ln}
    return tables


bacc.get_activation_tables = _patched_get_tables

# ---- problem constants (hardcoded; must match setup_inputs) ----
O, F, U, D = 4, 4, 8, 8
KH = KW = 3
NIJ = KH * KW
H = W = 64
C = 32
N_FULL = 16
N_CORES = 8
IMG_PER_CORE = N_FULL // N_CORES
HP, WP = H + 2, W + 2              # padded input
LT_ROWS = 2                        # output rows per 128-loc tile
NLT = H // LT_ROWS                 # 32 loc-tiles per image
ST_LT = 4                          # loc-tiles per super-tile (512 locs)
NST = NLT // ST_LT                 # 8 super-tiles per image
PB = 2                             # super-tiles batched per routing pass
PLT = PB * ST_LT                   # loc-tiles per routing pass (8)
KK = ST_LT * NIJ                   # collapsed (lt, ij) per half
NPP = NST // PB                    # routing passes per image
EPS = 1e-12

f32 = mybir.dt.float32
f16 = mybir.dt.float16
AL = mybir.AluOpType
AF = mybir.ActivationFunctionType
AX = mybir.AxisListType

_COMPILED = None
USE_POOL = True                    # it0 H-chain on the Pool engine (fp32)


def _build(dump=False, repeat=1):
    nc = bacc.Bacc("TRN2", target_bir_lowering=False, debug=False)

    # fp16 "pair" trick: priors matmuls contract over 96 partitions
    # (x_hi, x_lo, x_hi) x (w_hi, w_hi, w_lo), giving x*w exact to ~2^-22
    # at fp16 matmul speed (PE cycles depend only on output columns).
    xin_d = nc.dram_tensor("xin", [IMG_PER_CORE, 3 * C, H * W], f16,
                           kind="ExternalInput").ap()
    wmov_d = nc.dram_tensor("wmov", [3 * C, NIJ * 128], f16,
                            kind="ExternalInput").ap()
    wsum_d = nc.dram_tensor("wsum", [3 * C, NIJ * 32], f16,
                            kind="ExternalInput").ap()
    # output stays location-major [loc-tile, loc-in-tile, channel]; the
    # host transposes to channel-major during the gather step
    out_d = nc.dram_tensor("out", [IMG_PER_CORE, H * W, C], f32,
                           kind="ExternalOutput").ap()

    with tile.TileContext(nc) as tc:
        with tc.tile_pool(name="const", bufs=1) as cpool, \
             tc.tile_pool(name="xpad", bufs=1) as xpool, \
             tc.tile_pool(name="pst", bufs=6) as ppool, \
             tc.tile_pool(name="bpool", bufs=3) as bpool, \
             tc.tile_pool(name="gh", bufs=2) as ghpool, \
             tc.tile_pool(name="gh32", bufs=1) as gh32pool, \
             tc.tile_pool(name="small", bufs=2) as smpool, \
             tc.tile_pool(name="sm1", bufs=1) as sm1pool, \
             tc.tile_pool(name="ppri", bufs=2, space="PSUM") as ppri, \
             tc.tile_pool(name="ps0", bufs=2, space="PSUM") as ps0:

            wmov_s = cpool.tile([3 * C, NIJ * 128], f16, tag="wmov")
            wsum_s = cpool.tile([3 * C, NIJ * 32], f16, tag="wsum")
            eps_s = cpool.tile([128, 1], f32, tag="eps")
            nc.sync.dma_start(wmov_s[:], wmov_d[:])
            nc.sync.dma_start(wsum_s[:], wsum_d[:])
            nc.gpsimd.memset(eps_s[:], EPS)

            # both images' padded inputs are loaded up-front (no image-
            # boundary bubble in the software pipeline)
            xvs = []
            for img in range(IMG_PER_CORE):
                xp = xpool.tile([3 * C, HP * WP], f16, tag=f"xpad{img}",
                                name=f"xpad{img}")
                nc.gpsimd.memset(xp[:], 0.0)
                xv = xp[:].rearrange("p (h w) -> p h w", h=HP, w=WP)
                nc.sync.dma_start(
                    xv[:, 1:1 + H, 1:1 + W],
                    xin_d[img].rearrange("p (h w) -> p h w", h=H, w=W))
                xvs.append(xv)


            plist = [(img, pr % NPP) for img in range(IMG_PER_CORE)
                     for pr in range(NPP * repeat)]

            def P5(P_st):
                return P_st[:].rearrange("p (k o u f) -> p k o u f",
                                         k=KK, o=O, u=U, f=F)

            def bhalf(t, half):
                return t[:, half * ST_LT * 144:
                         (half + 1) * ST_LT * 144].rearrange(
                    "p (k o f) -> p k o f", k=KK, o=O, f=F)

            def squash(s_st, tagp, pool, vdt=f16, veng=None,
                       vexp_out=True):
                # s_st: [128, (lt, o, u)] fp32 or fp16.  Returns vexp
                # [128, (lt, o, u, f)] (v broadcast along f, ready as the
                # H-mult operand) or plain v [128, (lt, o, u)] f16.
                sq = smpool.tile([128, PLT * 32], f32, tag=f"sq{tagp}")
                nc.scalar.activation(sq[:], s_st[:], AF.Square)
                n2 = smpool.tile([128, PLT * O], f32, tag=f"n2{tagp}")
                nc.vector.tensor_reduce(
                    n2[:],
                    sq[:].rearrange("p (g u) -> p g u", g=PLT * O, u=U),
                    AX.X, AL.add)
                # t = sqrt(n2+eps) via exp(0.5*ln(.)); the ~5e-6 table
                # error is far below the fp16 noise floor (no Newton).
                Lt = smpool.tile([128, PLT * O], f32, tag=f"L{tagp}")
                nc.scalar.activation(Lt[:], n2[:], AF.Ln, bias=eps_s[:])
                t_ = smpool.tile([128, PLT * O], f32, tag=f"t{tagp}")
                nc.scalar.activation(t_[:], Lt[:], AF.Exp, scale=0.5)
                # w = (1+n2)*t;  fi = n2 / w
                pw = smpool.tile([128, PLT * O], f32, tag=f"pw{tagp}")
                nc.vector.scalar_tensor_tensor(
                    pw[:], n2[:], 1.0, t_[:], AL.add, AL.mult)
                rw = smpool.tile([128, PLT * O], f32, tag=f"rw{tagp}")
                nc.vector.reciprocal_approx_fast(rw[:], pw[:])
                fi = smpool.tile([128, PLT * O], f32, tag=f"fi{tagp}")
                nc.vector.tensor_tensor(fi[:], n2[:], rw[:], AL.mult)
                fib = fi[:].rearrange("p (lt o) -> p lt o",
                                      lt=PLT).unsqueeze(3)
                sv = s_st[:].rearrange("p (lt o u) -> p lt o u", lt=PLT,
                                       o=O, u=U)
                if not vexp_out:
                    v = pool.tile([128, PLT * 32], f32, tag=f"v{tagp}")
                    nc.vector.tensor_tensor(
                        v[:].rearrange("p (lt o u) -> p lt o u", lt=PLT,
                                       o=O, u=U),
                        sv, fib.broadcast_to((128, PLT, O, U)), AL.mult)
                    return v
                vexp = pool.tile([128, PLT * 128], vdt, tag=f"vx{tagp}")
                (veng or nc.vector).tensor_tensor(
                    vexp[:].rearrange("p (lt o u f) -> p lt o u f",
                                      lt=PLT, o=O, u=U, f=F),
                    sv.unsqueeze(4).broadcast_to((128, PLT, O, U, F)),
                    fib.unsqueeze(4).broadcast_to((128, PLT, O, U, F)),
                    AL.mult)
                return vexp

            def hmult_btree(P_st, vexp, half, b_out, eng, pool, hdt, htag):
                # H = P * v, then tree-reduce over u into b_out
                # [128, ST_LT*144] viewed [p, k, o, f].
                Hst = pool.tile([128, ST_LT * 1152], hdt, tag=htag)
                Hv = Hst[:].rearrange("p (lt ij c) -> p lt ij c",
                                      lt=ST_LT, ij=NIJ, c=128)
                vb = vexp[:, half * ST_LT * 128:
                          (half + 1) * ST_LT * 128].rearrange(
                    "p (lt c) -> p lt c",
                    lt=ST_LT).unsqueeze(2).broadcast_to(
                        (128, ST_LT, NIJ, 128))
                Pv = P_st[:].rearrange("p (lt ij c) -> p lt ij c",
                                       lt=ST_LT, ij=NIJ, c=128)
                eng.tensor_tensor(Hv, Pv, vb, AL.mult)
                Hk = Hst[:].rearrange("p (k o u f) -> p k o u f", k=KK,
                                      o=O, u=U, f=F)
                eng.tensor_tensor(
                    Hk[:, :, :, 0:4, :], Hk[:, :, :, 0:4, :],
                    Hk[:, :, :, 4:8, :], AL.add)
                eng.tensor_tensor(
                    Hk[:, :, :, 0:2, :], Hk[:, :, :, 0:2, :],
                    Hk[:, :, :, 2:4, :], AL.add)
                eng.tensor_tensor(
                    b_out.unsqueeze(3), Hk[:, :, :, 0:1, :],
                    Hk[:, :, :, 1:2, :], AL.add)

            def produce(idx):
                img, pr = plist[idx]
                xv = xvs[img]
                P_sts = []
                # all 8 loc-tiles' s0 accumulate into one PSUM tile;
                # squash0 reads it from PSUM directly (no ACT copy)
                s0_st = ps0.tile([128, PLT * 32], f32, tag="s0p")
                for half in range(PB):
                    st = pr * PB + half
                    P_st = ppool.tile([128, ST_LT * 1152], f16, tag="P")
                    P_sts.append(P_st)
                    for lt in range(ST_LT):
                        r0 = (st * ST_LT + lt) * LT_ROWS
                        glt = half * ST_LT + lt
                        pp = ppri.tile([128, 1152], f32, tag="ppri")
                        s0p = s0_st[:, glt * 32:(glt + 1) * 32]
                        for ij in range(NIJ):
                            i, j = ij // KW, ij % KW
                            for r in range(LT_ROWS):
                                xw = xv[:, r0 + i + r, j:j + W]
                                prow = slice(r * W, (r + 1) * W)
                                nc.tensor.matmul(
                                    pp[prow, ij * 128:(ij + 1) * 128],
                                    xw,
                                    wmov_s[:, ij * 128:(ij + 1) * 128],
                                    start=True, stop=True)
                                nc.tensor.matmul(
                                    s0p[prow], xw,
                                    wsum_s[:, ij * 32:(ij + 1) * 32],
                                    start=(ij == 0),
                                    stop=(ij == NIJ - 1))
                        nc.scalar.copy(
                            P_st[:, lt * 1152:(lt + 1) * 1152], pp[:])

                # it0: b1 = sum_u P * v0  (fp32, Pool engine)
                b_st = bpool.tile([128, PLT * 144], f32, tag="b")
                if USE_POOL:
                    vexp = squash(s0_st, "0", sm1pool, vdt=f32,
                                  veng=nc.gpsimd)
                    for half in range(PB):
                        hmult_btree(P_sts[half], vexp, half,
                                    bhalf(b_st, half), nc.gpsimd,
                                    gh32pool, f32, "gh32")
                else:
                    vexp = squash(s0_st, "0", sm1pool)
                    for half in range(PB):
                        hmult_btree(P_sts[half], vexp, half,
                                    bhalf(b_st, half), nc.vector,
                                    ghpool, f16, "gh")
                return P_sts, b_st

            def consume(idx, P_sts, b_st, its):
                img, pr = plist[idx]
                b2 = b_st if its == (2,) else None
                s_st = None
                for it in its:
                    # E = exp(b); Z = sum_o E; E2 = E / Z
                    E = smpool.tile([128, PLT * 144], f32, tag="E")
                    nc.scalar.activation(E[:], (b_st if it == 1 else
                                                b2)[:], AF.Exp)
                    Ev = E[:].rearrange("p (k o f) -> p k o f",
                                        k=PLT * NIJ, o=O, f=F)
                    Zt = sm1pool.tile([128, PLT * 72], f32, tag="Zt")
                    Ztv = Zt[:].rearrange("p (k g f) -> p k g f",
                                          k=PLT * NIJ, g=2, f=F)
                    nc.vector.tensor_tensor(
                        Ztv, Ev[:, :, 0:2, :], Ev[:, :, 2:4, :], AL.add)
                    Z = sm1pool.tile([128, PLT * 36], f32, tag="Z")
                    nc.vector.tensor_tensor(
                        Z[:].rearrange("p (k f) -> p k f", k=PLT * NIJ,
                                       f=F).unsqueeze(2),
                        Ztv[:, :, 0:1, :], Ztv[:, :, 1:2, :], AL.add)
                    rZ = sm1pool.tile([128, PLT * 36], f32, tag="rZ")
                    nc.vector.reciprocal_approx_fast(rZ[:], Z[:])
                    E2 = sm1pool.tile([128, PLT * 144], f16, tag="E2")
                    nc.vector.tensor_tensor(
                        E2[:].rearrange("p (k o f) -> p k o f",
                                        k=PLT * NIJ, o=O, f=F),
                        Ev,
                        rZ[:].rearrange("p (k f) -> p k f", k=PLT * NIJ,
                                        f=F).unsqueeze(2).broadcast_to(
                            (128, PLT * NIJ, O, F)),
                        AL.mult)

                    # G = E2 * P; s = sum_{ij,f} G
                    sf = sm1pool.tile([128, PLT * 64], f16, tag="sf")
                    for half in range(PB):
                        G = ghpool.tile([128, ST_LT * 1152], f16,
                                        tag="gh")
                        Gk = G[:].rearrange("p (k o u f) -> p k o u f",
                                            k=KK, o=O, u=U, f=F)
                        Eb = E2[:, half * ST_LT * 144:(half + 1) *
                                ST_LT * 144].rearrange(
                            "p (k o f) -> p k o f", k=KK,
                            o=O).unsqueeze(3).broadcast_to(
                                (128, KK, O, U, F))
                        nc.vector.tensor_tensor(Gk, P5(P_sts[half]), Eb,
                                                AL.mult)
                        # ij-tree: 9 = (0:4 += 4:8) -> (0:2 += 2:4)
                        #          -> (0 += 1) -> (0 += 8)
                        Gv = G[:].rearrange("p (lt ij c) -> p lt ij c",
                                            lt=ST_LT, ij=NIJ, c=128)
                        nc.vector.tensor_tensor(
                            Gv[:, :, 0:4, :], Gv[:, :, 0:4, :],
                            Gv[:, :, 4:8, :], AL.add)
                        nc.vector.tensor_tensor(
                            Gv[:, :, 0:2, :], Gv[:, :, 0:2, :],
                            Gv[:, :, 2:4, :], AL.add)
                        nc.vector.tensor_tensor(
                            Gv[:, :, 0:1, :], Gv[:, :, 0:1, :],
                            Gv[:, :, 1:2, :], AL.add)
                        nc.vector.tensor_tensor(
                            Gv[:, :, 0:1, :], Gv[:, :, 0:1, :],
                            Gv[:, :, 8:9, :], AL.add)
                        # f-tree step 1 into sf [p, lt, (o,u), g=2]
                        G0 = Gv[:, :, 0, :].rearrange(
                            "p lt (w f) -> p lt w f", w=32, f=F)
                        sfv = sf[:, half * ST_LT * 64:(half + 1) *
                                 ST_LT * 64].rearrange(
                            "p (lt w g) -> p lt w g", lt=ST_LT, w=32,
                            g=2)
                        nc.vector.tensor_tensor(
                            sfv, G0[:, :, :, 0:2], G0[:, :, :, 2:4],
                            AL.add)
                    s_st = sm1pool.tile([128, PLT * 32], f16, tag="s")
                    sfp = sf[:].rearrange("p (m g) -> p m g",
                                          m=PLT * 32, g=2)
                    nc.vector.tensor_tensor(
                        s_st[:].unsqueeze(2), sfp[:, :, 0:1],
                        sfp[:, :, 1:2], AL.add)

                    if it == 1:
                        # b2 = b1 + sum_u P * v1
                        vexp = squash(s_st, "1", sm1pool)
                        hred = sm1pool.tile([128, PLT * 144], f16,
                                            tag="hred")
                        for half in range(PB):
                            hmult_btree(P_sts[half], vexp, half,
                                        bhalf(hred, half), nc.vector,
                                        ghpool, f16, "gh")
                        b2 = sm1pool.tile([128, PLT * 144], f32,
                                          tag="b2")
                        nc.vector.tensor_tensor(b2[:], b_st[:], hred[:],
                                                AL.add)

                # output: v2 = squash(s2) fp32, DMA out location-major
                v = squash(s_st, "2", smpool, vexp_out=False)
                ov = out_d[img].rearrange("(lt p) c -> p lt c", lt=NLT,
                                          p=128)
                nc.sync.dma_start(
                    ov[:, pr * PLT:(pr + 1) * PLT, :],
                    v[:].rearrange("p (lt c) -> p lt c", lt=PLT, c=C))

            # software pipeline, depth 2.  produce(k+2) is emitted BETWEEN
            # routing iterations 1 and 2 of pass k so that (a) exp(b1) of
            # pass k isn't queued behind the ACT PSUM->SBUF copies of pass
            # k+2, and (b) PE priors run continuously ahead.
            queue = []
            for i in range(len(plist) + 2):
                if queue and queue[0][0] == i - 2:
                    idx, P_sts, b_st = queue[0]
                    b2 = consume(idx, P_sts, b_st, its=(1,))
                if i < len(plist):
                    queue.append((i,) + produce(i))
                if queue and queue[0][0] == i - 2:
                    idx, P_sts, b_st = queue.pop(0)
                    consume(idx, P_sts, b2, its=(2,))

    nc.compile()
    return nc


def _get_compiled():
    global _COMPILED
    if _COMPILED is None:
        _COMPILED = _build()
    return _COMPILED


def _hi_lo(a):
    hi = a.astype(np.float16)
    lo = (a - hi.astype(np.float32)).astype(np.float16)
    return hi, lo


def _stack3(mat):
    # [C, n] fp32 -> [3C, n] f16 rows (hi, lo, hi); pairs with the
    # (w_hi, w_hi, w_lo) weight stack for an exact-to-~2^-22 product
    hi, lo = _hi_lo(mat)
    return np.concatenate([hi, lo, hi], axis=0)


def _make_consts(weight):
    w = np.asarray(weight, dtype=np.float32)  # [o, f, i, j, u, d]
    wmov = np.zeros((C, NIJ * 128), dtype=np.float32)
    wsum = np.zeros((C, NIJ * 32), dtype=np.float32)
    for o in range(O):
        for f in range(F):
            for ij in range(NIJ):
                i, j = ij // KW, ij % KW
                for u in range(U):
                    for d in range(D):
                        wmov[f * D + d,
                             ij * 128 + o * 32 + u * 4 + f] = w[o, f, i, j,
                                                                u, d]
                        wsum[f * D + d,
                             ij * 32 + o * 8 + u] = 0.25 * w[o, f, i, j, u,
                                                             d]

    def wstack(m):
        hi, lo = _hi_lo(m)
        return np.concatenate([hi, hi, lo], axis=0)

    return wstack(wmov), wstack(wsum)


def from_device_layout(o):
    """[N, H*W, C] loc-major device output -> [N, C, H, W] fp32."""
    o = np.asarray(o, dtype=np.float32).reshape(-1, NLT, LT_ROWS, W, C)
    return np.ascontiguousarray(
        o.transpose(0, 4, 1, 2, 3)).reshape(-1, C, H, W)


def kernel(x, weight):
    x = np.ascontiguousarray(np.asarray(x, dtype=np.float32))
    wmov, wsum = _make_consts(weight)

    nc = _get_compiled()
    xs = np.stack([_stack3(img.reshape(C, H * W))
                   for img in x.reshape(N_FULL, C, H * W)])
    in_maps = []
    for c in range(N_CORES):
        in_maps.append({
            "xin": np.ascontiguousarray(
                xs[c * IMG_PER_CORE:(c + 1) * IMG_PER_CORE]),
            "wmov": wmov,
            "wsum": wsum,
        })
    res = bass_utils.run_bass_kernel_spmd(nc, in_maps,
                                          core_ids=list(range(N_CORES)))
    out = np.empty((N_FULL, C, H, W), dtype=np.float32)
    for c in range(N_CORES):
        out[c * IMG_PER_CORE:(c + 1) * IMG_PER_CORE] = from_device_layout(
            res.results[c]["out"])
    return out


# revision 44
# speedup vs baseline: 1.8544x; 1.8544x over previous
"""CapsuleConv2d (3-iteration dynamic routing) Bass kernel for 8 TRN2 cores.

Strategy (data-parallel over batch, 2 images per core):
  - priors[l, ij, o, u, f] computed by PE per 128-location tile in fp32
    (one merged-row matmul per kernel tap: strided stationary
    [32, 2 rows, 64] = 128 locs).  s0 = 0.25*sum P accumulated by PE in
    the same pass.
  - routing in "natural" layout (locations on partitions).  The big
    elementwise multiplies run on DVE in fp16 which engages the 2x_1p
    perf mode (2 elem/cycle/lane; requires every operand 2-byte with a
    packed last dim).  Segmented reductions are binary trees of
    tensor_tensor adds (also 2x in fp16) instead of tensor_reduce (which
    has no perf mode).  Softmax normalizers use the 1-cyc/elem
    reciprocal_approx_fast custom DVE op.
  - the iteration-0 H-chain (H = P*v0, tree-reduce to logits b1) runs on
    the Pool engine in fp32: removes its fp16 rounding (the dominant
    error term) AND its DVE load.  The kernel is software-pipelined:
    produce(k+1) = priors + squash0 + Pool chain is emitted before
    consume(k) = DVE routing, so Pool(k+1) overlaps DVE(k).
  - exp/ln/square and PSUM->SBUF copies on ACT; PE transposes the final
    [128 locs, 32 ch] result for channel-major DMA out.
"""
import numpy as np

import concourse.bass as bass
import concourse.bacc as bacc
import concourse.tile as tile
import concourse.mybir as mybir
import concourse.bass_utils as bass_utils

# All ACT functions we use (Exp, Ln, Square, Copy, ...) live together in the
# "natural_log_exp_and_others" table set, but bacc's table-load pass picks a
# per-function set greedily (Ln -> natural_log, Exp -> exp_and_others),
# thrashing ~2.7us table loads between them.  Restrict Exp/Ln to the combined
# set so a single load covers the whole kernel.
_orig_get_tables = bacc.get_activation_tables
_AFT = mybir.ActivationFunctionType


def _patched_get_tables(arch):
    tables = dict(_orig_get_tables(arch))
    for name, funcs in tables.items():
        if name != "natural_log_exp_and_others":
            tables[name] = funcs - {_AFT.Exp, _AFT.Ln}
    return tables


bacc.get_activation_tables = _patched_get_tables

# ---- problem constants (hardcoded; must match setup_inputs) ----
O, F, U, D = 4, 4, 8, 8
KH = KW = 3
NIJ = KH * KW
H = W = 64
C = 32
N_FULL = 16
N_CORES = 8
IMG_PER_CORE = N_FULL // N_CORES
HP, WP = H + 2, W + 2              # padded input
LT_ROWS = 2                        # output rows per 128-loc tile
NLT = H // LT_ROWS                 # 32 loc-tiles per image
ST_LT = 4                          # loc-tiles per super-tile (512 locs)
NST = NLT // ST_LT                 # 8 super-tiles per image
PB = 2                             # super-tiles batched per routing pass
PLT = PB * ST_LT                   # loc-tiles per routing pass (8)
KK = ST_LT * NIJ                   # collapsed (lt, ij) per half
NPP = NST // PB                    # routing passes per image
EPS = 1e-12

f32 = mybir.dt.float32
f16 = mybir.dt.float16
AL = mybir.AluOpType
AF = mybir.ActivationFunctionType
AX = mybir.AxisListType

_COMPILED = None
USE_POOL = False                    # it0 H-chain on the Pool engine (fp32)


def _build(dump=False, repeat=1):
    nc = bacc.Bacc("TRN2", target_bir_lowering=False, debug=False)

    # fp16 "pair" trick: priors matmuls contract over 96 partitions
    # (x_hi, x_lo, x_hi) x (w_hi, w_hi, w_lo), giving x*w exact to ~2^-22
    # at fp16 matmul speed (PE cycles depend only on output columns).
    xin_d = nc.dram_tensor("xin", [IMG_PER_CORE, 3 * C, H * W], f16,
                           kind="ExternalInput").ap()
    wmov_d = nc.dram_tensor("wmov", [3 * C, NIJ * 128], f16,
                            kind="ExternalInput").ap()
    wsum_d = nc.dram_tensor("wsum", [3 * C, NIJ * 32], f16,
                            kind="ExternalInput").ap()
    # output stays location-major [loc-tile, loc-in-tile, channel]; the
    # host transposes to channel-major during the gather step
    out_d = nc.dram_tensor("out", [IMG_PER_CORE, H * W, C], f32,
                           kind="ExternalOutput").ap()

    with tile.TileContext(nc) as tc:
        with tc.tile_pool(name="const", bufs=1) as cpool, \
             tc.tile_pool(name="xpad", bufs=1) as xpool, \
             tc.tile_pool(name="pst", bufs=6) as ppool, \
             tc.tile_pool(name="bpool", bufs=3) as bpool, \
             tc.tile_pool(name="gh", bufs=2) as ghpool, \
             tc.tile_pool(name="gh32", bufs=1) as gh32pool, \
             tc.tile_pool(name="small", bufs=2) as smpool, \
             tc.tile_pool(name="sm1", bufs=1) as sm1pool, \
             tc.tile_pool(name="ppri", bufs=2, space="PSUM") as ppri, \
             tc.tile_pool(name="ps0", bufs=2, space="PSUM") as ps0:

            wmov_s = cpool.tile([3 * C, NIJ * 128], f16, tag="wmov")
            wsum_s = cpool.tile([3 * C, NIJ * 32], f16, tag="wsum")
            eps_s = cpool.tile([128, 1], f32, tag="eps")
            nc.sync.dma_start(wmov_s[:], wmov_d[:])
            nc.sync.dma_start(wsum_s[:], wsum_d[:])
            nc.gpsimd.memset(eps_s[:], EPS)

            # both images' padded inputs are loaded up-front (no image-
            # boundary bubble in the software pipeline)
            xvs = []
            for img in range(IMG_PER_CORE):
                xp = xpool.tile([3 * C, HP * WP], f16, tag=f"xpad{img}",
                                name=f"xpad{img}")
                nc.gpsimd.memset(xp[:], 0.0)
                xv = xp[:].rearrange("p (h w) -> p h w", h=HP, w=WP)
                nc.sync.dma_start(
                    xv[:, 1:1 + H, 1:1 + W],
                    xin_d[img].rearrange("p (h w) -> p h w", h=H, w=W))
                xvs.append(xv)


            plist = [(img, pr % NPP) for img in range(IMG_PER_CORE)
                     for pr in range(NPP * repeat)]

            def P5(P_st):
                return P_st[:].rearrange("p (k o u f) -> p k o u f",
                                         k=KK, o=O, u=U, f=F)

            def bhalf(t, half):
                return t[:, half * ST_LT * 144:
                         (half + 1) * ST_LT * 144].rearrange(
                    "p (k o f) -> p k o f", k=KK, o=O, f=F)

            def squash(s_st, tagp, pool, vdt=f16, veng=None,
                       vexp_out=True):
                # s_st: [128, (lt, o, u)] fp32 or fp16.  Returns vexp
                # [128, (lt, o, u, f)] (v broadcast along f, ready as the
                # H-mult operand) or plain v [128, (lt, o, u)] f16.
                sq = smpool.tile([128, PLT * 32], f32, tag=f"sq{tagp}")
                nc.scalar.activation(sq[:], s_st[:], AF.Square)
                n2 = smpool.tile([128, PLT * O], f32, tag=f"n2{tagp}")
                nc.vector.tensor_reduce(
                    n2[:],
                    sq[:].rearrange("p (g u) -> p g u", g=PLT * O, u=U),
                    AX.X, AL.add)
                # t = sqrt(n2+eps) via exp(0.5*ln(.)); the ~5e-6 table
                # error is far below the fp16 noise floor (no Newton).
                Lt = smpool.tile([128, PLT * O], f32, tag=f"L{tagp}")
                nc.scalar.activation(Lt[:], n2[:], AF.Ln, bias=eps_s[:])
                t_ = smpool.tile([128, PLT * O], f32, tag=f"t{tagp}")
                nc.scalar.activation(t_[:], Lt[:], AF.Exp, scale=0.5)
                # w = (1+n2)*t;  fi = n2 / w
                pw = smpool.tile([128, PLT * O], f32, tag=f"pw{tagp}")
                nc.vector.scalar_tensor_tensor(
                    pw[:], n2[:], 1.0, t_[:], AL.add, AL.mult)
                rw = smpool.tile([128, PLT * O], f32, tag=f"rw{tagp}")
                nc.vector.reciprocal_approx_fast(rw[:], pw[:])
                fi = smpool.tile([128, PLT * O], f32, tag=f"fi{tagp}")
                nc.vector.tensor_tensor(fi[:], n2[:], rw[:], AL.mult)
                fib = fi[:].rearrange("p (lt o) -> p lt o",
                                      lt=PLT).unsqueeze(3)
                sv = s_st[:].rearrange("p (lt o u) -> p lt o u", lt=PLT,
                                       o=O, u=U)
                if not vexp_out:
                    v = pool.tile([128, PLT * 32], f32, tag=f"v{tagp}")
                    nc.vector.tensor_tensor(
                        v[:].rearrange("p (lt o u) -> p lt o u", lt=PLT,
                                       o=O, u=U),
                        sv, fib.broadcast_to((128, PLT, O, U)), AL.mult)
                    return v
                vexp = pool.tile([128, PLT * 128], vdt, tag=f"vx{tagp}")
                (veng or nc.vector).tensor_tensor(
                    vexp[:].rearrange("p (lt o u f) -> p lt o u f",
                                      lt=PLT, o=O, u=U, f=F),
                    sv.unsqueeze(4).broadcast_to((128, PLT, O, U, F)),
                    fib.unsqueeze(4).broadcast_to((128, PLT, O, U, F)),
                    AL.mult)
                return vexp

            def hmult_btree(P_st, vexp, half, b_out, eng, pool, hdt, htag):
                # H = P * v, then tree-reduce over u into b_out
                # [128, ST_LT*144] viewed [p, k, o, f].
                Hst = pool.tile([128, ST_LT * 1152], hdt, tag=htag)
                Hv = Hst[:].rearrange("p (lt ij c) -> p lt ij c",
                                      lt=ST_LT, ij=NIJ, c=128)
                vb = vexp[:, half * ST_LT * 128:
                          (half + 1) * ST_LT * 128].rearrange(
                    "p (lt c) -> p lt c",
                    lt=ST_LT).unsqueeze(2).broadcast_to(
                        (128, ST_LT, NIJ, 128))
                Pv = P_st[:].rearrange("p (lt ij c) -> p lt ij c",
                                       lt=ST_LT, ij=NIJ, c=128)
                eng.tensor_tensor(Hv, Pv, vb, AL.mult)
                Hk = Hst[:].rearrange("p (k o u f) -> p k o u f", k=KK,
                                      o=O, u=U, f=F)
                eng.tensor_tensor(
                    Hk[:, :, :, 0:4, :], Hk[:, :, :, 0:4, :],
                    Hk[:, :, :, 4:8, :], AL.add)
                eng.tensor_tensor(
                    Hk[:, :, :, 0:2, :], Hk[:, :, :, 0:2, :],
                    Hk[:, :, :, 2:4, :], AL.add)
                eng.tensor_tensor(
                    b_out.unsqueeze(3), Hk[:, :, :, 0:1, :],
                    Hk[:, :, :, 1:2, :], AL.add)

            def produce(idx):
                img, pr = plist[idx]
                xv = xvs[img]
                P_sts = []
                # all 8 loc-tiles' s0 accumulate into one PSUM tile;
                # squash0 reads it from PSUM directly (no ACT copy)
                s0_st = ps0.tile([128, PLT * 32], f32, tag="s0p")
                for half in range(PB):
                    st = pr * PB + half
                    P_st = ppool.tile([128, ST_LT * 1152], f16, tag="P")
                    P_sts.append(P_st)
                    for lt in range(ST_LT):
                        r0 = (st * ST_LT + lt) * LT_ROWS
                        glt = half * ST_LT + lt
                        pp = ppri.tile([128, 1152], f32, tag="ppri")
                        s0p = s0_st[:, glt * 32:(glt + 1) * 32]
                        for ij in range(NIJ):
                            i, j = ij // KW, ij % KW
                            for r in range(LT_ROWS):
                                xw = xv[:, r0 + i + r, j:j + W]
                                prow = slice(r * W, (r + 1) * W)
                                nc.tensor.matmul(
                                    pp[prow, ij * 128:(ij + 1) * 128],
                                    xw,
                                    wmov_s[:, ij * 128:(ij + 1) * 128],
                                    start=True, stop=True)
                                nc.tensor.matmul(
                                    s0p[prow], xw,
                                    wsum_s[:, ij * 32:(ij + 1) * 32],
                                    start=(ij == 0),
                                    stop=(ij == NIJ - 1))
                        nc.scalar.copy(
                            P_st[:, lt * 1152:(lt + 1) * 1152], pp[:])

                # it0: b1 = sum_u P * v0  (fp32, Pool engine)
                b_st = bpool.tile([128, PLT * 144], f32, tag="b")
                if USE_POOL:
                    vexp = squash(s0_st, "0", sm1pool, vdt=f32,
                                  veng=nc.gpsimd)
                    for half in range(PB):
                        hmult_btree(P_sts[half], vexp, half,
                                    bhalf(b_st, half), nc.gpsimd,
                                    gh32pool, f32, "gh32")
                else:
                    vexp = squash(s0_st, "0", sm1pool)
                    for half in range(PB):
                        hmult_btree(P_sts[half], vexp, half,
                                    bhalf(b_st, half), nc.vector,
                                    ghpool, f16, "gh")
                return P_sts, b_st

            def consume(idx, P_sts, b_st, its):
                img, pr = plist[idx]
                b2 = b_st if its == (2,) else None
                s_st = None
                for it in its:
                    # E = exp(b); Z = sum_o E; E2 = E / Z
                    E = smpool.tile([128, PLT * 144], f32, tag="E")
                    nc.scalar.activation(E[:], (b_st if it == 1 else
                                                b2)[:], AF.Exp)
                    Ev = E[:].rearrange("p (k o f) -> p k o f",
                                        k=PLT * NIJ, o=O, f=F)
                    Zt = sm1pool.tile([128, PLT * 72], f32, tag="Zt")
                    Ztv = Zt[:].rearrange("p (k g f) -> p k g f",
                                          k=PLT * NIJ, g=2, f=F)
                    nc.vector.tensor_tensor(
                        Ztv, Ev[:, :, 0:2, :], Ev[:, :, 2:4, :], AL.add)
                    Z = sm1pool.tile([128, PLT * 36], f32, tag="Z")
                    nc.vector.tensor_tensor(
                        Z[:].rearrange("p (k f) -> p k f", k=PLT * NIJ,
                                       f=F).unsqueeze(2),
                        Ztv[:, :, 0:1, :], Ztv[:, :, 1:2, :], AL.add)
                    rZ = sm1pool.tile([128, PLT * 36], f32, tag="rZ")
                    nc.vector.reciprocal_approx_fast(rZ[:], Z[:])
                    E2 = sm1pool.tile([128, PLT * 144], f16, tag="E2")
                    nc.vector.tensor_tensor(
                        E2[:].rearrange("p (k o f) -> p k o f",
                                        k=PLT * NIJ, o=O, f=F),
                        Ev,
                        rZ[:].rearrange("p (k f) -> p k f", k=PLT * NIJ,
                                        f=F).unsqueeze(2).broadcast_to(
                            (128, PLT * NIJ, O, F)),
                        AL.mult)

                    # G = E2 * P; s = sum_{ij,f} G
                    sf = sm1pool.tile([128, PLT * 64], f16, tag="sf")
                    for half in range(PB):
                        G = ghpool.tile([128, ST_LT * 1152], f16,
                                        tag="gh")
                        Gk = G[:].rearrange("p (k o u f) -> p k o u f",
                                            k=KK, o=O, u=U, f=F)
                        Eb = E2[:, half * ST_LT * 144:(half + 1) *
                                ST_LT * 144].rearrange(
                            "p (k o f) -> p k o f", k=KK,
                            o=O).unsqueeze(3).broadcast_to(
                                (128, KK, O, U, F))
                        nc.vector.tensor_tensor(Gk, P5(P_sts[half]), Eb,
                                                AL.mult)
                        # ij-tree: 9 = (0:4 += 4:8) -> (0:2 += 2:4)
                        #          -> (0 += 1) -> (0 += 8)
                        Gv = G[:].rearrange("p (lt ij c) -> p lt ij c",
                                            lt=ST_LT, ij=NIJ, c=128)
                        nc.vector.tensor_tensor(
                            Gv[:, :, 0:4, :], Gv[:, :, 0:4, :],
                            Gv[:, :, 4:8, :], AL.add)
                        nc.vector.tensor_tensor(
                            Gv[:, :, 0:2, :], Gv[:, :, 0:2, :],
                            Gv[:, :, 2:4, :], AL.add)
                        nc.vector.tensor_tensor(
                            Gv[:, :, 0:1, :], Gv[:, :, 0:1, :],
                            Gv[:, :, 1:2, :], AL.add)
                        nc.vector.tensor_tensor(
                            Gv[:, :, 0:1, :], Gv[:, :, 0:1, :],
                            Gv[:, :, 8:9, :], AL.add)
                        # f-tree step 1 into sf [p, lt, (o,u), g=2]
                        G0 = Gv[:, :, 0, :].rearrange(
                            "p lt (w f) -> p lt w f", w=32, f=F)
                        sfv = sf[:, half * ST_LT * 64:(half + 1) *
                                 ST_LT * 64].rearrange(
                            "p (lt w g) -> p lt w g", lt=ST_LT, w=32,
                            g=2)
                        nc.vector.tensor_tensor(
                            sfv, G0[:, :, :, 0:2], G0[:, :, :, 2:4],
                            AL.add)
                    s_st = sm1pool.tile([128, PLT * 32], f16, tag="s")
                    sfp = sf[:].rearrange("p (m g) -> p m g",
                                          m=PLT * 32, g=2)
                    nc.vector.tensor_tensor(
                        s_st[:].unsqueeze(2), sfp[:, :, 0:1],
                        sfp[:, :, 1:2], AL.add)

                    if it == 1:
                        # b2 = b1 + sum_u P * v1
                        vexp = squash(s_st, "1", sm1pool)
                        hred = sm1pool.tile([128, PLT * 144], f16,
                                            tag="hred")
                        for half in range(PB):
                            hmult_btree(P_sts[half], vexp, half,
                                        bhalf(hred, half), nc.vector,
                                        ghpool, f16, "gh")
                        b2 = sm1pool.tile([128, PLT * 144], f32,
                                          tag="b2")
                        nc.vector.tensor_tensor(b2[:], b_st[:], hred[:],
                                                AL.add)
                if its == (1,):
                    return b2

                # output: v2 = squash(s2) fp32, DMA out location-major
                v = squash(s_st, "2", smpool, vexp_out=False)
                ov = out_d[img].rearrange("(lt p) c -> p lt c", lt=NLT,
                                          p=128)
                nc.sync.dma_start(
                    ov[:, pr * PLT:(pr + 1) * PLT, :],
                    v[:].rearrange("p (lt c) -> p lt c", lt=PLT, c=C))

            # software pipeline, depth 2.  produce(k+2) is emitted BETWEEN
            # routing iterations 1 and 2 of pass k so that (a) exp(b1) of
            # pass k isn't queued behind the ACT PSUM->SBUF copies of pass
            # k+2, and (b) PE priors run continuously ahead.
            queue = []
            for i in range(len(plist) + 2):
                if queue and queue[0][0] == i - 2:
                    idx, P_sts, b_st = queue[0]
                    b2 = consume(idx, P_sts, b_st, its=(1,))
                if i < len(plist):
                    queue.append((i,) + produce(i))
                if queue and queue[0][0] == i - 2:
                    idx, P_sts, b_st = queue.pop(0)
                    consume(idx, P_sts, b2, its=(2,))

    nc.compile()
    return nc


def _get_compiled():
    global _COMPILED
    if _COMPILED is None:
        _COMPILED = _build()
    return _COMPILED


def _hi_lo(a):
    hi = a.astype(np.float16)
    lo = (a - hi.astype(np.float32)).astype(np.float16)
    return hi, lo


def _stack3(mat):
    # [C, n] fp32 -> [3C, n] f16 rows (hi, lo, hi); pairs with the
    # (w_hi, w_hi, w_lo) weight stack for an exact-to-~2^-22 product
    hi, lo = _hi_lo(mat)
    return np.concatenate([hi, lo, hi], axis=0)


def _make_consts(weight):
    w = np.asarray(weight, dtype=np.float32)  # [o, f, i, j, u, d]
    wmov = np.zeros((C, NIJ * 128), dtype=np.float32)
    wsum = np.zeros((C, NIJ * 32), dtype=np.float32)
    for o in range(O):
        for f in range(F):
            for ij in range(NIJ):
                i, j = ij // KW, ij % KW
                for u in range(U):
                    for d in range(D):
                        wmov[f * D + d,
                             ij * 128 + o * 32 + u * 4 + f] = w[o, f, i, j,
                                                                u, d]
                        wsum[f * D + d,
                             ij * 32 + o * 8 + u] = 0.25 * w[o, f, i, j, u,
                                                             d]

    def wstack(m):
        hi, lo = _hi_lo(m)
        return np.concatenate([hi, hi, lo], axis=0)

    return wstack(wmov), wstack(wsum)


def from_device_layout(o):
    """[N, H*W, C] loc-major device output -> [N, C, H, W] fp32."""
    o = np.asarray(o, dtype=np.float32).reshape(-1, NLT, LT_ROWS, W, C)
    return np.ascontiguousarray(
        o.transpose(0, 4, 1, 2, 3)).reshape(-1, C, H, W)


def kernel(x, weight):
    x = np.ascontiguousarray(np.asarray(x, dtype=np.float32))
    wmov, wsum = _make_consts(weight)

    nc = _get_compiled()
    xs = np.stack([_stack3(img.reshape(C, H * W))
                   for img in x.reshape(N_FULL, C, H * W)])
    in_maps = []
    for c in range(N_CORES):
        in_maps.append({
            "xin": np.ascontiguousarray(
                xs[c * IMG_PER_CORE:(c + 1) * IMG_PER_CORE]),
            "wmov": wmov,
            "wsum": wsum,
        })
    res = bass_utils.run_bass_kernel_spmd(nc, in_maps,
                                          core_ids=list(range(N_CORES)))
    out = np.empty((N_FULL, C, H, W), dtype=np.float32)
    for c in range(N_CORES):
        out[c * IMG_PER_CORE:(c + 1) * IMG_PER_CORE] = from_device_layout(
            res.results[c]["out"])
    return out
